# revision 16
# baseline (speedup 1.0000x reference)
"""GAT (2-layer graph attention network) TRN2 Bass kernel, 8-core SPMD.

Strategy (sharding_hint): shard the NxN attention row-wise (query dim) across
8 cores; replicate parameters; each core computes its 768-row block of both
attention layers. Layer-1 output blocks are AllGathered (as transposed blocks)
so every core can form the full [N, 256] hidden for layer 2.

Math trick: exp(leaky_relu(z)) == max(exp(z), exp(0.2*z)), and
exp(0.2*z) = exp(0.2*f1_i) * exp(0.2*f2_j) factorizes rank-1, so the masked
attention numerator is p = adj * max(exp(f1_i + f2_j), C_i * D_j), built with
one ACT pass + two DVE ops per tile in the *transposed* layout [j, i] that
feeds the PE aggregation matmul directly (no transposes of the big matrix).
Row softmax denominators come free via a ones-column in the aggregation lhsT.
"""
import sys

for _p in ("/opt/trn_rl_repo", "/root/.axon_site/_ro/trn_rl_repo"):
    if _p not in sys.path:
        sys.path.insert(0, _p)

import numpy as np
import ml_dtypes

from concourse import bacc, mybir, tile, masks
from concourse import bass_utils

# problem dims (hardcoded per harness contract)
N, F, H, Fh, C = 6144, 512, 4, 64, 40
NC = 8
R = N // NC          # 768 rows per core
T = N // 128         # 48 j-tiles
KT = F // 128        # 4 k-tiles of features
JC = N // 512        # 12 column chunks for streaming matmuls
HT_K = H * Fh        # 256 hidden dim
RH = R // 2          # half-block (agg matmul moving chunk, <= 512)
IB = R // 128        # 128-row sub-blocks per core block
ALPHA = 0.2

f32 = mybir.dt.float32
f32r = mybir.dt.float32r
bf16 = mybir.dt.bfloat16
AF = mybir.ActivationFunctionType
OP = mybir.AluOpType

# elementwise dtype for the big [N, R] tiles ("bf16" or "f32")
EW_DT = bf16


def MM_DT():
    # matmul operand dtype: fp32r runs at 1 cyc/row (vs 4 for fp32) when the
    # moving dim is >= 256, with near-fp32 precision
    return bf16 if EW_DT == bf16 else f32r


def _mm(nc, out, lhsT, rhs, **kw):
    nc.tensor.matmul(out, lhsT, rhs, **kw)


def attention_rows(tc, nc, pools, adjT_sb, lhsT_ext, m_cols, f1b, Cb, f2col, Dcol,
                   psum_pool, tag):
    """One attention 'head': builds p tiles [128j, R] for all T j-tiles and
    accumulates agg = lhsT_ext.T @ p into two psum chunks [m_cols, 384].

    f1b:  [128, R] f32   f1 (block rows) broadcast over partitions
    Cb:   [128, R] ew    exp(0.2*f1block) broadcast
    f2col:[128, 1] f32 slice per j-tile (AP function of J)
    Dcol: [128, 1] ew  slice per j-tile
    Returns (psA, psB) psum tiles [m_cols, 384] each.
    """
    psA = psum_pool.tile([128, RH], f32, tag="aggA", name="psA")[0:m_cols, :]
    psB = psum_pool.tile([128, RH], f32, tag="aggB", name="psB")[0:m_cols, :]
    upool, tpool, ppool = pools
    for J in range(T):
        u1 = upool.tile([128, R], EW_DT, tag="u1")
        # u1 = exp(f1_i + f2_j)   (unmasked; masked by final mul)
        nc.scalar.activation(u1[:], f1b[:], AF.Exp, bias=f2col(J), scale=1.0)
        # t = max(C_i * D_j, u1)
        t = tpool.tile([128, R], EW_DT, tag="t")
        nc.vector.scalar_tensor_tensor(t[:], Cb[:], Dcol(J), u1[:],
                                       OP.mult, OP.max)
        # p = t * adjT
        p = ppool.tile([128, R], MM_DT(), tag="p")
        nc.vector.tensor_tensor(p[:], t[:], adjT_sb[:, J, :], OP.mult)
        # aggregate: psum += lhsT_ext.T @ p
        _mm(nc, psA[:], lhsT_ext(J), p[:, 0:RH],
            start=(J == 0), stop=(J == T - 1))
        _mm(nc, psB[:], lhsT_ext(J), p[:, RH:R],
            start=(J == 0), stop=(J == T - 1))
    return psA, psB


def epilogue_block(tc, nc, sbuf, psum_pool, ident, psA, psB, m_cols, emit):
    """Per 128-row sub-block: transpose agg psum [m_cols, 384] chunks back to
    row-major [128, m_cols], normalize by the sums column, apply ELU, then
    call emit(ib, elu_tile[128, m_cols-1])."""
    nv = m_cols - 1
    for chunk, ps in ((0, psA), (1, psB)):
        hu = sbuf.tile([m_cols, RH], f32, tag="ep_hu")
        nc.scalar.copy(hu[:], ps[:])
        for s in range(RH // 128):
            ib = chunk * (RH // 128) + s
            ptp = psum_pool.tile([128, 128], f32, tag="ep_tp",
                                 name="ptp")[:, 0:m_cols]
            nc.tensor.transpose(ptp[:], hu[:, s * 128:(s + 1) * 128],
                                ident[0:m_cols, 0:m_cols])
            hi = sbuf.tile([128, m_cols], f32, tag="ep_hi")
            nc.vector.tensor_copy(hi[:], ptp[:])
            sinv = sbuf.tile([128, 1], f32, tag="ep_sinv")
            nc.vector.reciprocal(sinv[:], hi[:, nv:nv + 1])
            # r = relu(h/s), m = min(h/s, 0)
            r = sbuf.tile([128, nv], f32, tag="ep_r")
            nc.vector.tensor_scalar(r[:], hi[:, 0:nv], sinv[:], 0.0,
                                    OP.mult, OP.max)
            m = sbuf.tile([128, nv], f32, tag="ep_m")
            nc.vector.tensor_scalar(m[:], hi[:, 0:nv], sinv[:], 0.0,
                                    OP.mult, OP.min)
            e = sbuf.tile([128, nv], f32, tag="ep_e")
            nc.scalar.activation(e[:], m[:], AF.Exp)
            elu = sbuf.tile([128, nv], f32, tag="ep_elu")
            # elu = (r - 1) + e
            nc.vector.scalar_tensor_tensor(elu[:], r[:], -1.0, e[:],
                                           OP.add, OP.add)
            emit(ib, elu)


def build_nc(reps=1):
    nc = bacc.Bacc("TRN2", target_bir_lowering=False, debug=False,
                   num_devices=NC)

    adjT_d = nc.dram_tensor("adjT", [N, R], EW_DT, kind="ExternalInput")
    xT_d = nc.dram_tensor("xT", [F, N], f32, kind="ExternalInput")
    xTb_d = nc.dram_tensor("xTblk", [F, R], f32, kind="ExternalInput")
    W_d = nc.dram_tensor("W", [H, F, Fh], f32, kind="ExternalInput")
    a1_d = nc.dram_tensor("a1", [1, H * Fh], f32, kind="ExternalInput")
    a2_d = nc.dram_tensor("a2", [1, H * Fh], f32, kind="ExternalInput")
    Wo_d = nc.dram_tensor("Wo", [HT_K, C], f32, kind="ExternalInput")
    ao1_d = nc.dram_tensor("ao1", [1, C], f32, kind="ExternalInput")
    ao2_d = nc.dram_tensor("ao2", [1, C], f32, kind="ExternalInput")
    y_d = nc.dram_tensor("y", [R, C], f32, kind="ExternalOutput")

    with tile.TileContext(nc) as tc:
        for r in range(reps):
            gat_body(tc, nc, adjT_d, xT_d, xTb_d, W_d, a1_d, a2_d, Wo_d, ao1_d,
                     ao2_d, y_d, rep=r)
    nc.compile()
    return nc


def gat_body(tc, nc, adjT_d, xT_d, xTb_d, W_d, a1_d, a2_d, Wo_d, ao1_d, ao2_d,
             y_d, rep=0):
    with tc.tile_pool(name=f"persist{rep}", bufs=1) as persist, \
         tc.tile_pool(name=f"ew_u{rep}", bufs=2) as upool, \
         tc.tile_pool(name=f"ew_t{rep}", bufs=2) as tpool, \
         tc.tile_pool(name=f"ew_p{rep}", bufs=4) as ppool, \
         tc.tile_pool(name=f"bcast{rep}", bufs=2) as bpool, \
         tc.tile_pool(name=f"small{rep}", bufs=2) as small, \
         tc.tile_pool(name=f"psum_agg{rep}", bufs=1, space="PSUM") as psum_agg, \
         tc.tile_pool(name=f"psum_sm{rep}", bufs=2, space="PSUM") as psum_sm, \
         tc.tile_pool(name=f"dram{rep}", bufs=1, space="DRAM") as dram:

        ew_pools = (upool, tpool, ppool)
        ident = persist.tile([128, 128], f32)
        masks.make_identity(nc, ident[:])

        # ---- adjT load + convert to EW_DT (resident all phases) ----
        adjT_sb = persist.tile([128, T, R], EW_DT)
        for J in range(T):
            nc.sync.dma_start(adjT_sb[:, J, :],
                              adjT_d.ap()[J * 128:(J + 1) * 128, :])

        fdram = dram.tile([2 * H, N], f32)
        fdram2 = dram.tile([2, N], f32)
        ag_in = dram.tile([HT_K, R], f32r)
        ag_out = dram.tile([NC * HT_K, R], f32r, addr_space="Shared")

        with tc.tile_pool(name=f"l1{rep}", bufs=1) as l1, \
             tc.tile_pool(name=f"psum_l1{rep}", bufs=1, space="PSUM") as psum_l1:

            Whe = l1.tile([128, H, T, Fh + 1], MM_DT())
            nc.vector.memset(Whe[:, :, :, Fh:Fh + 1], 1.0)
            fblk = l1.tile([2 * H, R], f32)

            with tc.tile_pool(name=f"ph1_{rep}", bufs=1) as ph1, \
                 tc.tile_pool(name=f"xts{rep}", bufs=2) as xts, \
                 tc.tile_pool(name=f"fsb{rep}", bufs=2) as fsbp:
                # ---- parameter prep ----
                W_all = ph1.tile([128, KT, H, Fh], f32)
                W_v = W_d.ap().rearrange("h (kt p) f -> kt p h f", p=128)
                for kt in range(KT):
                    nc.sync.dma_start(W_all[:, kt, :, :], W_v[kt])
                a12 = ph1.tile([1, 2, HT_K], f32)
                nc.sync.dma_start(a12[:, 0, :], a1_d.ap())
                nc.sync.dma_start(a12[:, 1, :], a2_d.ap())
                a12b = ph1.tile([128, 2, HT_K], f32)
                nc.gpsimd.partition_broadcast(a12b[:], a12[:])
                W_allr = ph1.tile([128, KT, H * Fh], f32r)
                nc.vector.tensor_copy(
                    W_allr[:], W_all[:].rearrange("p k h f -> p k (h f)"))
                # wv[:, kt, v] : v in 0..3 -> W_h @ a1_h ; 4..7 -> W_h @ a2_h
                wv = ph1.tile([128, KT, 2 * H], f32r)
                for v in range(2):
                    for kt in range(KT):
                        wtmp = xts.tile([128, H, Fh], f32, tag="wtmp")
                        nc.vector.tensor_tensor(
                            wtmp[:], W_all[:, kt, :, :],
                            a12b[:, v, :].rearrange("p (h f) -> p h f", h=H),
                            OP.mult)
                        with nc.allow_low_precision(reason="f32r reduce"):
                            nc.vector.tensor_reduce(
                                wv[:, kt, v * H:(v + 1) * H], wtmp[:],
                                mybir.AxisListType.X, OP.add)

                # ---- stream xT: f-vectors (flat) + Wh (=> Whe lhsT ext) ----
                NJ2 = 256
                for jc in range(N // NJ2):
                    xt = xts.tile([128, KT, NJ2], f32, tag="xt")
                    for kt in range(KT):
                        nc.sync.dma_start(
                            xt[:, kt, :],
                            xT_d.ap()[kt * 128:(kt + 1) * 128,
                                      jc * NJ2:(jc + 1) * NJ2])
                    xtr = xts.tile([128, KT, NJ2], f32r, tag="xtr")
                    nc.vector.tensor_copy(xtr[:], xt[:])
                    psf = psum_l1.tile([2 * H, NJ2], f32, tag="psf")
                    for kt in range(KT):
                        _mm(nc, psf[:], wv[:, kt, :], xtr[:, kt, :],
                            start=(kt == 0), stop=(kt == KT - 1))
                    fsb = fsbp.tile([2 * H, NJ2], f32, tag="fsb")
                    nc.scalar.copy(fsb[:], psf[:])
                    nc.sync.dma_start(fdram[:, jc * NJ2:(jc + 1) * NJ2], fsb[:])
                    for js in range(NJ2 // 128):
                        Jt = jc * (NJ2 // 128) + js
                        psw = psum_l1.tile([128, H * Fh], f32, tag="psw", bufs=2)
                        for kt in range(KT):
                            _mm(nc, psw[:], xtr[:, kt, js * 128:(js + 1) * 128],
                                W_allr[:, kt, :],
                                start=(kt == 0), stop=(kt == KT - 1))
                        nc.vector.tensor_copy(
                            Whe[:, :, Jt, 0:Fh],
                            psw[:].rearrange("p (h f) -> p h f", h=H))

                # ---- f1 block values (from xTblk; per-core rows block) ----
                psfb_a = psum_l1.tile([2 * H, RH], f32, tag="psf")
                psfb_b = psum_l1.tile([2 * H, RH], f32, tag="psfb")
                for kt in range(KT):
                    xtb = xts.tile([128, R], f32, tag="xtb")
                    nc.sync.dma_start(xtb[:],
                                      xTb_d.ap()[kt * 128:(kt + 1) * 128, :])
                    xtbr = xts.tile([128, R], f32r, tag="xtbr")
                    nc.vector.tensor_copy(xtbr[:], xtb[:])
                    _mm(nc, psfb_a[:], wv[:, kt, :], xtbr[:, 0:RH],
                        start=(kt == 0), stop=(kt == KT - 1))
                    _mm(nc, psfb_b[:], wv[:, kt, :], xtbr[:, RH:R],
                        start=(kt == 0), stop=(kt == KT - 1))
                nc.scalar.copy(fblk[:, 0:RH], psfb_a[:])
                nc.scalar.copy(fblk[:, RH:R], psfb_b[:])
            cexp = l1.tile([H, R], EW_DT)
            nc.scalar.activation(cexp[:], fblk[0:H, :], AF.Exp, scale=ALPHA)

            # ---- per-head score columns ----
            f2col_all = l1.tile([128, H, T], f32)
            nc.sync.dma_start(
                f2col_all[:],
                fdram[H:2 * H, :].rearrange("h (t p) -> p h t", p=128))
            Dcol_all = l1.tile([128, H, T], f32)
            nc.scalar.activation(Dcol_all[:], f2col_all[:], AF.Exp, scale=ALPHA)

            # ---- layer-1 heads ----
            h_sb = l1.tile([128, IB, HT_K], f32)
            for h in range(H):
                stg1 = bpool.tile([1, R], f32, tag="stg1", bufs=1)
                nc.sync.dma_start(stg1[:], fblk[h:h + 1, :])
                f1b = bpool.tile([128, R], f32, tag="f1b")
                nc.gpsimd.partition_broadcast(f1b[:], stg1[:])
                stg2 = bpool.tile([1, R], EW_DT, tag="stg2", bufs=1)
                nc.sync.dma_start(stg2[:], cexp[h:h + 1, :])
                Cb = bpool.tile([128, R], EW_DT, tag="Cb")
                nc.gpsimd.partition_broadcast(Cb[:], stg2[:])
                psA, psB = attention_rows(
                    tc, nc, ew_pools, adjT_sb,
                    lambda J, h=h: Whe[:, h, J, :],
                    Fh + 1, f1b, Cb,
                    lambda J, h=h: f2col_all[:, h, J:J + 1],
                    lambda J, h=h: Dcol_all[:, h, J:J + 1],
                    psum_agg, tag="l1")

                def emit_h(ib, elu, h=h):
                    nc.vector.tensor_copy(
                        h_sb[:, ib, h * Fh:(h + 1) * Fh], elu[:])
                epilogue_block(tc, nc, small, psum_sm, ident, psA, psB,
                               Fh + 1, emit_h)

            # ---- transpose h block -> hTblk, send to AllGather ----
            hTblk = l1.tile([128, 2, R], f32r)
            for ib in range(IB):
                for kt in range(2):
                    ptp = psum_sm.tile([128, 128], f32, tag="ep_tp")
                    nc.tensor.transpose(
                        ptp[:], h_sb[:, ib, kt * 128:(kt + 1) * 128], ident[:])
                    nc.vector.tensor_copy(
                        hTblk[:, kt, ib * 128:(ib + 1) * 128], ptp[:])
            for kt in range(2):
                nc.sync.dma_start(ag_in[kt * 128:(kt + 1) * 128, :],
                                  hTblk[:, kt, :])

            # ---- layer-2 block-f vectors (local, pre-gather) ----
            Wo_sb = l1.tile([128, 2, C], f32)
            nc.sync.dma_start(Wo_sb[:],
                              Wo_d.ap().rearrange("(kt p) c -> p kt c", p=128))
            ao = l1.tile([1, 2, C], f32)
            nc.sync.dma_start(ao[:, 0, :], ao1_d.ap())
            nc.sync.dma_start(ao[:, 1, :], ao2_d.ap())
            aob = l1.tile([128, 2, C], f32)
            nc.gpsimd.partition_broadcast(aob[:], ao[:])
            wov = l1.tile([128, 2, 2], f32r)
            wotmp = l1.tile([128, 2, C], f32)
            for v in range(2):
                for kt in range(2):
                    nc.vector.tensor_tensor(wotmp[:, kt, :], Wo_sb[:, kt, :],
                                            aob[:, v, :], OP.mult)
                with nc.allow_low_precision(reason="f32r reduce"):
                    nc.vector.tensor_reduce(wov[:, :, v:v + 1], wotmp[:],
                                            mybir.AxisListType.X, OP.add)
            psfb2a = psum_l1.tile([2, RH], f32, tag="psf")
            psfb2b = psum_l1.tile([2, RH], f32, tag="psfb")
            for kt in range(2):
                _mm(nc, psfb2a[:], wov[:, kt, :], hTblk[:, kt, 0:RH],
                    start=(kt == 0), stop=(kt == 1))
                _mm(nc, psfb2b[:], wov[:, kt, :], hTblk[:, kt, RH:R],
                    start=(kt == 0), stop=(kt == 1))
            fblk2 = l1.tile([2, R], f32)
            nc.scalar.copy(fblk2[:, 0:RH], psfb2a[:])
            nc.scalar.copy(fblk2[:, RH:R], psfb2b[:])
            cexp2 = l1.tile([1, R], EW_DT)
            nc.scalar.activation(cexp2[:], fblk2[0:1, :], AF.Exp, scale=ALPHA)
            f1ob = bpool.tile([128, R], f32, tag="f1b")
            nc.gpsimd.partition_broadcast(f1ob[:], fblk2[0:1, :])
            Cob = bpool.tile([128, R], EW_DT, tag="Cb")
            nc.gpsimd.partition_broadcast(Cob[:], cexp2[:])

            # keep small params needed after l1 pool closes
            wov_keep = persist.tile([128, 2, 2], f32r)
            nc.vector.tensor_copy(wov_keep[:], wov[:])
            Wo_keep = persist.tile([128, 2, C], f32r)
            nc.vector.tensor_copy(Wo_keep[:], Wo_sb[:])

        # ---- AllGather hT blocks ----
        nc.gpsimd.collective_compute(
            "AllGather", OP.bypass,
            replica_groups=[list(range(NC))],
            ins=[ag_in[:]], outs=[ag_out[:]],
        )

        with tc.tile_pool(name=f"l2{rep}", bufs=1) as l2, \
             tc.tile_pool(name=f"psum_l2{rep}", bufs=1, space="PSUM") as psum_l2:
            hTf = l2.tile([128, 2, N], f32r)
            agv = ag_out[:].rearrange("(r k) i -> k r i", r=NC)
            for kt in range(2):
                nc.sync.dma_start(
                    hTf[:, kt, :].rearrange("p (r i) -> p r i", r=NC),
                    agv[kt * 128:(kt + 1) * 128, :, :])

            # Who = h @ Wo (+ones col) and f2o flat
            Whoe = l2.tile([128, T, C + 1], MM_DT())
            nc.vector.memset(Whoe[:, :, C:C + 1], 1.0)
            for jc in range(JC):
                psf2 = psum_l2.tile([2, 512], f32, tag="psf2")
                for kt in range(2):
                    _mm(nc, psf2[:], wov_keep[:, kt, :],
                        hTf[:, kt, jc * 512:(jc + 1) * 512],
                        start=(kt == 0), stop=(kt == 1))
                fsb2 = small.tile([2, 512], f32, tag="fsb2")
                nc.scalar.copy(fsb2[:], psf2[:])
                nc.sync.dma_start(fdram2[:, jc * 512:(jc + 1) * 512], fsb2[:])
                for js in range(4):
                    Jt = jc * 4 + js
                    psw = psum_l2.tile([128, C], f32, tag="psw2", bufs=2)
                    for kt in range(2):
                        _mm(nc, psw[:],
                            hTf[:, kt, Jt * 128:(Jt + 1) * 128],
                            Wo_keep[:, kt, :],
                            start=(kt == 0), stop=(kt == 1))
                    nc.vector.tensor_copy(Whoe[:, Jt, 0:C], psw[:])

            f2ocol = l2.tile([128, T], f32)
            nc.sync.dma_start(
                f2ocol[:],
                fdram2[1:2, :].rearrange("o (t p) -> p (o t)", p=128))
            Docol = l2.tile([128, T], f32)
            nc.scalar.activation(Docol[:], f2ocol[:], AF.Exp, scale=ALPHA)

            # ---- layer-2 attention + output ----
            psA, psB = attention_rows(
                tc, nc, ew_pools, adjT_sb,
                lambda J: Whoe[:, J, :],
                C + 1, f1ob, Cob,
                lambda J: f2ocol[:, J:J + 1],
                lambda J: Docol[:, J:J + 1],
                psum_agg, tag="l2")

            def emit_out(ib, elu):
                negm = small.tile([128, 1], f32, tag="o_negm")
                nc.vector.tensor_reduce(negm[:], elu[:], mybir.AxisListType.X,
                                        OP.max, negate=True)
                ex = small.tile([128, C], f32, tag="o_ex")
                s2 = small.tile([128, 1], f32, tag="o_s2")
                nc.scalar.activation(ex[:], elu[:], AF.Exp, bias=negm[:],
                                     accum_out=s2[:])
                ls = small.tile([128, 1], f32, tag="o_ls")
                nc.scalar.activation(ls[:], s2[:], AF.Ln)
                out = small.tile([128, C], f32, tag="o_out")
                nc.vector.tensor_scalar(out[:], elu[:], negm[:], ls[:],
                                        OP.add, OP.subtract)
                nc.sync.dma_start(y_d.ap()[ib * 128:(ib + 1) * 128, :], out[:])
            epilogue_block(tc, nc, small, psum_sm, ident, psA, psB,
                           C + 1, emit_out)


_NC_CACHE = {}


def _get_nc():
    if "nc" not in _NC_CACHE:
        _NC_CACHE["nc"] = build_nc()
    return _NC_CACHE["nc"]


def _make_in_maps(ins):
    x = np.asarray(ins["x"], np.float32)
    adj = np.asarray(ins["adj"], np.float32)
    xT = np.ascontiguousarray(x.T)
    adjT = np.ascontiguousarray(adj.T)  # [N, N]; cols are query rows
    adt = ml_dtypes.bfloat16 if EW_DT == bf16 else np.float32
    in_maps = []
    for c in range(NC):
        blk = slice(c * R, (c + 1) * R)
        in_maps.append({
            "adjT": np.ascontiguousarray(adjT[:, blk]).astype(adt),
            "xT": xT,
            "xTblk": np.ascontiguousarray(xT[:, blk]),
            "W": np.asarray(ins["W"], np.float32),
            "a1": np.asarray(ins["a1"], np.float32).reshape(1, H * Fh),
            "a2": np.asarray(ins["a2"], np.float32).reshape(1, H * Fh),
            "Wo": np.asarray(ins["Wo"], np.float32),
            "ao1": np.asarray(ins["ao1"], np.float32).reshape(1, C),
            "ao2": np.asarray(ins["ao2"], np.float32).reshape(1, C),
        })
    return in_maps


def kernel(x, adj, W, a1, a2, Wo, ao1, ao2):
    nc = _get_nc()
    in_maps = _make_in_maps(dict(x=x, adj=adj, W=W, a1=a1, a2=a2, Wo=Wo,
                                 ao1=ao1, ao2=ao2))
    res = bass_utils.run_bass_kernel_spmd(nc, in_maps, core_ids=list(range(NC)))
    return np.concatenate([res.results[c]["y"] for c in range(NC)], axis=0)


# revision 17
# speedup vs baseline: 375.0056x; 375.0056x over previous
"""GAT (2-layer graph attention network) TRN2 Bass kernel, 8-core SPMD.

Strategy (sharding_hint): shard the NxN attention row-wise (query dim) across
8 cores; replicate parameters; each core computes its 768-row block of both
attention layers. Layer-1 output blocks are AllGathered (as transposed blocks)
so every core can form the full [N, 256] hidden for layer 2.

Math trick: exp(leaky_relu(z)) == max(exp(z), exp(0.2*z)), and
exp(0.2*z) = exp(0.2*f1_i) * exp(0.2*f2_j) factorizes rank-1, so the masked
attention numerator is p = adj * max(exp(f1_i + f2_j), C_i * D_j), built with
one ACT pass + two DVE ops per tile in the *transposed* layout [j, i] that
feeds the PE aggregation matmul directly (no transposes of the big matrix).
Row softmax denominators come free via a ones-column in the aggregation lhsT.
"""
import sys

for _p in ("/opt/trn_rl_repo", "/root/.axon_site/_ro/trn_rl_repo"):
    if _p not in sys.path:
        sys.path.insert(0, _p)

import numpy as np
import ml_dtypes

from concourse import bacc, mybir, tile, masks
from concourse import bass_utils

# problem dims (hardcoded per harness contract)
N, F, H, Fh, C = 6144, 512, 4, 64, 40
NC = 8
R = N // NC          # 768 rows per core
T = N // 128         # 48 j-tiles
KT = F // 128        # 4 k-tiles of features
JC = N // 512        # 12 column chunks for streaming matmuls
HT_K = H * Fh        # 256 hidden dim
RH = R // 2          # half-block (agg matmul moving chunk, <= 512)
IB = R // 128        # 128-row sub-blocks per core block
ALPHA = 0.2

f32 = mybir.dt.float32
f32r = mybir.dt.float32r
bf16 = mybir.dt.bfloat16
AF = mybir.ActivationFunctionType
OP = mybir.AluOpType

# elementwise dtype for the big [N, R] tiles ("bf16" or "f32")
EW_DT = bf16
MOCK_AG = False


def MM_DT():
    # matmul operand dtype: fp32r runs at 1 cyc/row (vs 4 for fp32) when the
    # moving dim is >= 256, with near-fp32 precision
    return bf16 if EW_DT == bf16 else f32r


def _mm(nc, out, lhsT, rhs, **kw):
    nc.tensor.matmul(out, lhsT, rhs, **kw)


def attention_rows(tc, nc, pools, adjT_sb, lhsT_ext, m_cols, f1b, Cb, f2col, Dcol,
                   psum_pool, tag):
    """One attention 'head': builds p tiles [128j, R] for all T j-tiles and
    accumulates agg = lhsT_ext.T @ p into two psum chunks [m_cols, 384].

    f1b:  [128, R] f32   f1 (block rows) broadcast over partitions
    Cb:   [128, R] ew    exp(0.2*f1block) broadcast
    f2col:[128, 1] f32 slice per j-tile (AP function of J)
    Dcol: [128, 1] ew  slice per j-tile
    Returns (psA, psB) psum tiles [m_cols, 384] each.
    """
    psA = psum_pool.tile([128, RH], f32, tag="aggA", name="psA")[0:m_cols, :]
    psB = psum_pool.tile([128, RH], f32, tag="aggB", name="psB")[0:m_cols, :]
    upool, tpool, ppool = pools
    for J in range(T):
        u1 = upool.tile([128, R], EW_DT, tag="u1")
        # u1 = exp(f1_i + f2_j)   (unmasked; masked by final mul)
        nc.scalar.activation(u1[:], f1b[:], AF.Exp, bias=f2col(J), scale=1.0)
        # t = max(C_i * D_j, u1)
        t = tpool.tile([128, R], EW_DT, tag="t")
        nc.vector.scalar_tensor_tensor(t[:], Cb[:], Dcol(J), u1[:],
                                       OP.mult, OP.max)
        # p = t * adjT
        p = ppool.tile([128, R], MM_DT(), tag="p")
        nc.vector.tensor_tensor(p[:], t[:], adjT_sb[:, J, :], OP.mult)
        # aggregate: psum += lhsT_ext.T @ p
        _mm(nc, psA[:], lhsT_ext(J), p[:, 0:RH],
            start=(J == 0), stop=(J == T - 1))
        _mm(nc, psB[:], lhsT_ext(J), p[:, RH:R],
            start=(J == 0), stop=(J == T - 1))
    return psA, psB


def epilogue_block(tc, nc, sbuf, psum_pool, ident, psA, psB, m_cols, emit):
    """Per 128-row sub-block: transpose agg psum [m_cols, 384] chunks back to
    row-major [128, m_cols], normalize by the sums column, apply ELU, then
    call emit(ib, elu_tile[128, m_cols-1])."""
    nv = m_cols - 1
    for chunk, ps in ((0, psA), (1, psB)):
        hu = sbuf.tile([m_cols, RH], f32, tag="ep_hu")
        nc.scalar.copy(hu[:], ps[:])
        for s in range(RH // 128):
            ib = chunk * (RH // 128) + s
            ptp = psum_pool.tile([128, 128], f32, tag="ep_tp",
                                 name="ptp")[:, 0:m_cols]
            nc.tensor.transpose(ptp[:], hu[:, s * 128:(s + 1) * 128],
                                ident[0:m_cols, 0:m_cols])
            hi = sbuf.tile([128, m_cols], f32, tag="ep_hi")
            nc.vector.tensor_copy(hi[:], ptp[:])
            sinv = sbuf.tile([128, 1], f32, tag="ep_sinv")
            nc.vector.reciprocal(sinv[:], hi[:, nv:nv + 1])
            # r = relu(h/s), m = min(h/s, 0)
            r = sbuf.tile([128, nv], f32, tag="ep_r")
            nc.vector.tensor_scalar(r[:], hi[:, 0:nv], sinv[:], 0.0,
                                    OP.mult, OP.max)
            m = sbuf.tile([128, nv], f32, tag="ep_m")
            nc.vector.tensor_scalar(m[:], hi[:, 0:nv], sinv[:], 0.0,
                                    OP.mult, OP.min)
            e = sbuf.tile([128, nv], f32, tag="ep_e")
            nc.scalar.activation(e[:], m[:], AF.Exp)
            elu = sbuf.tile([128, nv], f32, tag="ep_elu")
            # elu = (r - 1) + e
            nc.vector.scalar_tensor_tensor(elu[:], r[:], -1.0, e[:],
                                           OP.add, OP.add)
            emit(ib, elu)


def build_nc(reps=1):
    nc = bacc.Bacc("TRN2", target_bir_lowering=False, debug=False,
                   num_devices=NC)

    adjT_d = nc.dram_tensor("adjT", [N, R], EW_DT, kind="ExternalInput")
    xT_d = nc.dram_tensor("xT", [F, N], f32, kind="ExternalInput")
    xTb_d = nc.dram_tensor("xTblk", [F, R], f32, kind="ExternalInput")
    W_d = nc.dram_tensor("W", [H, F, Fh], f32, kind="ExternalInput")
    a1_d = nc.dram_tensor("a1", [1, H * Fh], f32, kind="ExternalInput")
    a2_d = nc.dram_tensor("a2", [1, H * Fh], f32, kind="ExternalInput")
    Wo_d = nc.dram_tensor("Wo", [HT_K, C], f32, kind="ExternalInput")
    ao1_d = nc.dram_tensor("ao1", [1, C], f32, kind="ExternalInput")
    ao2_d = nc.dram_tensor("ao2", [1, C], f32, kind="ExternalInput")
    y_d = nc.dram_tensor("y", [R, C], f32, kind="ExternalOutput")

    with tile.TileContext(nc) as tc:
        for r in range(reps):
            gat_body(tc, nc, adjT_d, xT_d, xTb_d, W_d, a1_d, a2_d, Wo_d, ao1_d,
                     ao2_d, y_d, rep=r)
    nc.compile()
    return nc


def gat_body(tc, nc, adjT_d, xT_d, xTb_d, W_d, a1_d, a2_d, Wo_d, ao1_d, ao2_d,
             y_d, rep=0):
    with tc.tile_pool(name=f"persist{rep}", bufs=1) as persist, \
         tc.tile_pool(name=f"ew_u{rep}", bufs=2) as upool, \
         tc.tile_pool(name=f"ew_t{rep}", bufs=2) as tpool, \
         tc.tile_pool(name=f"ew_p{rep}", bufs=4) as ppool, \
         tc.tile_pool(name=f"bcast{rep}", bufs=2) as bpool, \
         tc.tile_pool(name=f"small{rep}", bufs=2) as small, \
         tc.tile_pool(name=f"psum_agg{rep}", bufs=1, space="PSUM") as psum_agg, \
         tc.tile_pool(name=f"psum_sm{rep}", bufs=2, space="PSUM") as psum_sm, \
         tc.tile_pool(name=f"dram{rep}", bufs=1, space="DRAM") as dram:

        ew_pools = (upool, tpool, ppool)
        ident = persist.tile([128, 128], f32)
        masks.make_identity(nc, ident[:])

        # ---- adjT load + convert to EW_DT (resident all phases) ----
        adjT_sb = persist.tile([128, T, R], EW_DT)
        for J in range(T):
            nc.sync.dma_start(adjT_sb[:, J, :],
                              adjT_d.ap()[J * 128:(J + 1) * 128, :])

        fdram = dram.tile([2 * H, N], f32)
        fdram2 = dram.tile([2, N], f32)
        ag_in = dram.tile([HT_K, R], f32r)
        ag_out = dram.tile([NC * HT_K, R], f32r, addr_space="Shared")

        with tc.tile_pool(name=f"l1{rep}", bufs=1) as l1, \
             tc.tile_pool(name=f"psum_l1{rep}", bufs=1, space="PSUM") as psum_l1:

            Whe = l1.tile([128, H, T, Fh + 1], MM_DT())
            nc.vector.memset(Whe[:, :, :, Fh:Fh + 1], 1.0)
            fblk = l1.tile([2 * H, R], f32)

            with tc.tile_pool(name=f"ph1_{rep}", bufs=1) as ph1, \
                 tc.tile_pool(name=f"xts{rep}", bufs=2) as xts, \
                 tc.tile_pool(name=f"fsb{rep}", bufs=2) as fsbp:
                # ---- parameter prep ----
                W_all = ph1.tile([128, KT, H, Fh], f32)
                W_v = W_d.ap().rearrange("h (kt p) f -> kt p h f", p=128)
                for kt in range(KT):
                    nc.sync.dma_start(W_all[:, kt, :, :], W_v[kt])
                a12 = ph1.tile([1, 2, HT_K], f32)
                nc.sync.dma_start(a12[:, 0, :], a1_d.ap())
                nc.sync.dma_start(a12[:, 1, :], a2_d.ap())
                a12b = ph1.tile([128, 2, HT_K], f32)
                nc.gpsimd.partition_broadcast(a12b[:], a12[:])
                W_allr = ph1.tile([128, KT, H * Fh], f32r)
                nc.vector.tensor_copy(
                    W_allr[:], W_all[:].rearrange("p k h f -> p k (h f)"))
                # wv[:, kt, v] : v in 0..3 -> W_h @ a1_h ; 4..7 -> W_h @ a2_h
                wv = ph1.tile([128, KT, 2 * H], f32r)
                for v in range(2):
                    for kt in range(KT):
                        wtmp = xts.tile([128, H, Fh], f32, tag="wtmp")
                        nc.vector.tensor_tensor(
                            wtmp[:], W_all[:, kt, :, :],
                            a12b[:, v, :].rearrange("p (h f) -> p h f", h=H),
                            OP.mult)
                        with nc.allow_low_precision(reason="f32r reduce"):
                            nc.vector.tensor_reduce(
                                wv[:, kt, v * H:(v + 1) * H], wtmp[:],
                                mybir.AxisListType.X, OP.add)

                # ---- stream xT: f-vectors (flat) + Wh (=> Whe lhsT ext) ----
                NJ2 = 256
                for jc in range(N // NJ2):
                    xt = xts.tile([128, KT, NJ2], f32, tag="xt")
                    for kt in range(KT):
                        nc.sync.dma_start(
                            xt[:, kt, :],
                            xT_d.ap()[kt * 128:(kt + 1) * 128,
                                      jc * NJ2:(jc + 1) * NJ2])
                    xtr = xts.tile([128, KT, NJ2], f32r, tag="xtr")
                    nc.vector.tensor_copy(xtr[:], xt[:])
                    psf = psum_l1.tile([2 * H, NJ2], f32, tag="psf")
                    for kt in range(KT):
                        _mm(nc, psf[:], wv[:, kt, :], xtr[:, kt, :],
                            start=(kt == 0), stop=(kt == KT - 1))
                    fsb = fsbp.tile([2 * H, NJ2], f32, tag="fsb")
                    nc.scalar.copy(fsb[:], psf[:])
                    nc.sync.dma_start(fdram[:, jc * NJ2:(jc + 1) * NJ2], fsb[:])
                    for js in range(NJ2 // 128):
                        Jt = jc * (NJ2 // 128) + js
                        psw = psum_l1.tile([128, H * Fh], f32, tag="psw", bufs=2)
                        for kt in range(KT):
                            _mm(nc, psw[:], xtr[:, kt, js * 128:(js + 1) * 128],
                                W_allr[:, kt, :],
                                start=(kt == 0), stop=(kt == KT - 1))
                        nc.vector.tensor_copy(
                            Whe[:, :, Jt, 0:Fh],
                            psw[:].rearrange("p (h f) -> p h f", h=H))

                # ---- f1 block values (from xTblk; per-core rows block) ----
                psfb_a = psum_l1.tile([2 * H, RH], f32, tag="psf")
                psfb_b = psum_l1.tile([2 * H, RH], f32, tag="psfb")
                for kt in range(KT):
                    xtb = xts.tile([128, R], f32, tag="xtb")
                    nc.sync.dma_start(xtb[:],
                                      xTb_d.ap()[kt * 128:(kt + 1) * 128, :])
                    xtbr = xts.tile([128, R], f32r, tag="xtbr")
                    nc.vector.tensor_copy(xtbr[:], xtb[:])
                    _mm(nc, psfb_a[:], wv[:, kt, :], xtbr[:, 0:RH],
                        start=(kt == 0), stop=(kt == KT - 1))
                    _mm(nc, psfb_b[:], wv[:, kt, :], xtbr[:, RH:R],
                        start=(kt == 0), stop=(kt == KT - 1))
                nc.scalar.copy(fblk[:, 0:RH], psfb_a[:])
                nc.scalar.copy(fblk[:, RH:R], psfb_b[:])
            cexp = l1.tile([H, R], EW_DT)
            nc.scalar.activation(cexp[:], fblk[0:H, :], AF.Exp, scale=ALPHA)

            # ---- per-head score columns ----
            f2col_all = l1.tile([128, H, T], f32)
            nc.sync.dma_start(
                f2col_all[:],
                fdram[H:2 * H, :].rearrange("h (t p) -> p h t", p=128))
            Dcol_all = l1.tile([128, H, T], f32)
            nc.scalar.activation(Dcol_all[:], f2col_all[:], AF.Exp, scale=ALPHA)

            # ---- layer-1 heads ----
            h_sb = l1.tile([128, IB, HT_K], f32)
            for h in range(H):
                stg1 = bpool.tile([1, R], f32, tag="stg1", bufs=1)
                nc.sync.dma_start(stg1[:], fblk[h:h + 1, :])
                f1b = bpool.tile([128, R], f32, tag="f1b")
                nc.gpsimd.partition_broadcast(f1b[:], stg1[:])
                stg2 = bpool.tile([1, R], EW_DT, tag="stg2", bufs=1)
                nc.sync.dma_start(stg2[:], cexp[h:h + 1, :])
                Cb = bpool.tile([128, R], EW_DT, tag="Cb")
                nc.gpsimd.partition_broadcast(Cb[:], stg2[:])
                psA, psB = attention_rows(
                    tc, nc, ew_pools, adjT_sb,
                    lambda J, h=h: Whe[:, h, J, :],
                    Fh + 1, f1b, Cb,
                    lambda J, h=h: f2col_all[:, h, J:J + 1],
                    lambda J, h=h: Dcol_all[:, h, J:J + 1],
                    psum_agg, tag="l1")

                def emit_h(ib, elu, h=h):
                    nc.vector.tensor_copy(
                        h_sb[:, ib, h * Fh:(h + 1) * Fh], elu[:])
                epilogue_block(tc, nc, small, psum_sm, ident, psA, psB,
                               Fh + 1, emit_h)

            # ---- transpose h block -> hTblk, send to AllGather ----
            hTblk = l1.tile([128, 2, R], f32r)
            for ib in range(IB):
                for kt in range(2):
                    ptp = psum_sm.tile([128, 128], f32, tag="ep_tp")
                    nc.tensor.transpose(
                        ptp[:], h_sb[:, ib, kt * 128:(kt + 1) * 128], ident[:])
                    nc.vector.tensor_copy(
                        hTblk[:, kt, ib * 128:(ib + 1) * 128], ptp[:])
            for kt in range(2):
                nc.sync.dma_start(ag_in[kt * 128:(kt + 1) * 128, :],
                                  hTblk[:, kt, :])

            # ---- layer-2 block-f vectors (local, pre-gather) ----
            Wo_sb = l1.tile([128, 2, C], f32)
            nc.sync.dma_start(Wo_sb[:],
                              Wo_d.ap().rearrange("(kt p) c -> p kt c", p=128))
            ao = l1.tile([1, 2, C], f32)
            nc.sync.dma_start(ao[:, 0, :], ao1_d.ap())
            nc.sync.dma_start(ao[:, 1, :], ao2_d.ap())
            aob = l1.tile([128, 2, C], f32)
            nc.gpsimd.partition_broadcast(aob[:], ao[:])
            wov = l1.tile([128, 2, 2], f32r)
            wotmp = l1.tile([128, 2, C], f32)
            for v in range(2):
                for kt in range(2):
                    nc.vector.tensor_tensor(wotmp[:, kt, :], Wo_sb[:, kt, :],
                                            aob[:, v, :], OP.mult)
                with nc.allow_low_precision(reason="f32r reduce"):
                    nc.vector.tensor_reduce(wov[:, :, v:v + 1], wotmp[:],
                                            mybir.AxisListType.X, OP.add)
            psfb2a = psum_l1.tile([2, RH], f32, tag="psf")
            psfb2b = psum_l1.tile([2, RH], f32, tag="psfb")
            for kt in range(2):
                _mm(nc, psfb2a[:], wov[:, kt, :], hTblk[:, kt, 0:RH],
                    start=(kt == 0), stop=(kt == 1))
                _mm(nc, psfb2b[:], wov[:, kt, :], hTblk[:, kt, RH:R],
                    start=(kt == 0), stop=(kt == 1))
            fblk2 = l1.tile([2, R], f32)
            nc.scalar.copy(fblk2[:, 0:RH], psfb2a[:])
            nc.scalar.copy(fblk2[:, RH:R], psfb2b[:])
            cexp2 = l1.tile([1, R], EW_DT)
            nc.scalar.activation(cexp2[:], fblk2[0:1, :], AF.Exp, scale=ALPHA)
            f1ob = bpool.tile([128, R], f32, tag="f1b")
            nc.gpsimd.partition_broadcast(f1ob[:], fblk2[0:1, :])
            Cob = bpool.tile([128, R], EW_DT, tag="Cb")
            nc.gpsimd.partition_broadcast(Cob[:], cexp2[:])

            # keep small params needed after l1 pool closes
            wov_keep = persist.tile([128, 2, 2], f32r)
            nc.vector.tensor_copy(wov_keep[:], wov[:])
            Wo_keep = persist.tile([128, 2, C], f32r)
            nc.vector.tensor_copy(Wo_keep[:], Wo_sb[:])

        # ---- AllGather hT blocks ----
        if MOCK_AG:
            # timing-model variant (no collectives): copy local block only
            nc.sync.dma_start(ag_out[0:HT_K, :], ag_in[:])
        else:
            nc.gpsimd.collective_compute(
                "AllGather", OP.bypass,
                replica_groups=[list(range(NC))],
                ins=[ag_in[:]], outs=[ag_out[:]],
            )

        with tc.tile_pool(name=f"l2{rep}", bufs=1) as l2, \
             tc.tile_pool(name=f"psum_l2{rep}", bufs=1, space="PSUM") as psum_l2:
            hTf = l2.tile([128, 2, N], f32r)
            agv = ag_out[:].rearrange("(r k) i -> k r i", r=NC)
            for kt in range(2):
                nc.sync.dma_start(
                    hTf[:, kt, :].rearrange("p (r i) -> p r i", r=NC),
                    agv[kt * 128:(kt + 1) * 128, :, :])

            # Who = h @ Wo (+ones col) and f2o flat
            Whoe = l2.tile([128, T, C + 1], MM_DT())
            nc.vector.memset(Whoe[:, :, C:C + 1], 1.0)
            for jc in range(JC):
                psf2 = psum_l2.tile([2, 512], f32, tag="psf2")
                for kt in range(2):
                    _mm(nc, psf2[:], wov_keep[:, kt, :],
                        hTf[:, kt, jc * 512:(jc + 1) * 512],
                        start=(kt == 0), stop=(kt == 1))
                fsb2 = small.tile([2, 512], f32, tag="fsb2")
                nc.scalar.copy(fsb2[:], psf2[:])
                nc.sync.dma_start(fdram2[:, jc * 512:(jc + 1) * 512], fsb2[:])
                for js in range(4):
                    Jt = jc * 4 + js
                    psw = psum_l2.tile([128, C], f32, tag="psw2", bufs=2)
                    for kt in range(2):
                        _mm(nc, psw[:],
                            hTf[:, kt, Jt * 128:(Jt + 1) * 128],
                            Wo_keep[:, kt, :],
                            start=(kt == 0), stop=(kt == 1))
                    nc.vector.tensor_copy(Whoe[:, Jt, 0:C], psw[:])

            f2ocol = l2.tile([128, T], f32)
            nc.sync.dma_start(
                f2ocol[:],
                fdram2[1:2, :].rearrange("o (t p) -> p (o t)", p=128))
            Docol = l2.tile([128, T], f32)
            nc.scalar.activation(Docol[:], f2ocol[:], AF.Exp, scale=ALPHA)

            # ---- layer-2 attention + output ----
            psA, psB = attention_rows(
                tc, nc, ew_pools, adjT_sb,
                lambda J: Whoe[:, J, :],
                C + 1, f1ob, Cob,
                lambda J: f2ocol[:, J:J + 1],
                lambda J: Docol[:, J:J + 1],
                psum_agg, tag="l2")

            def emit_out(ib, elu):
                negm = small.tile([128, 1], f32, tag="o_negm")
                nc.vector.tensor_reduce(negm[:], elu[:], mybir.AxisListType.X,
                                        OP.max, negate=True)
                ex = small.tile([128, C], f32, tag="o_ex")
                s2 = small.tile([128, 1], f32, tag="o_s2")
                nc.scalar.activation(ex[:], elu[:], AF.Exp, bias=negm[:],
                                     accum_out=s2[:])
                ls = small.tile([128, 1], f32, tag="o_ls")
                nc.scalar.activation(ls[:], s2[:], AF.Ln)
                out = small.tile([128, C], f32, tag="o_out")
                nc.vector.tensor_scalar(out[:], elu[:], negm[:], ls[:],
                                        OP.add, OP.subtract)
                nc.sync.dma_start(y_d.ap()[ib * 128:(ib + 1) * 128, :], out[:])
            epilogue_block(tc, nc, small, psum_sm, ident, psA, psB,
                           C + 1, emit_out)


_NC_CACHE = {}


def _get_nc():
    if "nc" not in _NC_CACHE:
        _NC_CACHE["nc"] = build_nc()
    return _NC_CACHE["nc"]


def _make_in_maps(ins):
    x = np.asarray(ins["x"], np.float32)
    adj = np.asarray(ins["adj"], np.float32)
    xT = np.ascontiguousarray(x.T)
    adjT = np.ascontiguousarray(adj.T)  # [N, N]; cols are query rows
    adt = ml_dtypes.bfloat16 if EW_DT == bf16 else np.float32
    in_maps = []
    for c in range(NC):
        blk = slice(c * R, (c + 1) * R)
        in_maps.append({
            "adjT": np.ascontiguousarray(adjT[:, blk]).astype(adt),
            "xT": xT,
            "xTblk": np.ascontiguousarray(xT[:, blk]),
            "W": np.asarray(ins["W"], np.float32),
            "a1": np.asarray(ins["a1"], np.float32).reshape(1, H * Fh),
            "a2": np.asarray(ins["a2"], np.float32).reshape(1, H * Fh),
            "Wo": np.asarray(ins["Wo"], np.float32),
            "ao1": np.asarray(ins["ao1"], np.float32).reshape(1, C),
            "ao2": np.asarray(ins["ao2"], np.float32).reshape(1, C),
        })
    return in_maps


def kernel(x, adj, W, a1, a2, Wo, ao1, ao2):
    nc = _get_nc()
    in_maps = _make_in_maps(dict(x=x, adj=adj, W=W, a1=a1, a2=a2, Wo=Wo,
                                 ao1=ao1, ao2=ao2))
    res = bass_utils.run_bass_kernel_spmd(nc, in_maps, core_ids=list(range(NC)))
    return np.concatenate([res.results[c]["y"] for c in range(NC)], axis=0)
